# revision 1
# baseline (speedup 1.0000x reference)
"""Trainium2 Bass kernel for nn_MultiHeadAttention_69930657513858.

Single-token (decode) multi-head attention, B=8, E=4096, H=32 heads of
D=128, with a KV cache that is identically ones (length L=4095).

Because the cache is all-ones, attention collapses to a closed form:
  scores = [s0]*L ++ [s1],  s0 = sum_d(q)/sqrt(D), s1 = (q.k)/sqrt(D)
  softmax => p_last = sigmoid(s1 - s0 - ln(L)); cache mass = 1 - p_last
  o = (1 - p_last)*ones + p_last*v = 1 + p_last*(v - 1)
so the kernel is four GEMMs (q,k,v projections + out-proj) plus O(B*H)
scalar work.  Sharding: tensor-parallel over heads, 4 heads per core
(Wq/Wk/Wv row-sliced, Wo column-sliced); partial out-proj results are
summed on the host (the "all-reduce").

The PE contracts over the partition dim, so each GEMM needs the weight's
in_features on partitions; weight slices are pre-transposed on the host
while sharding (same class of host work as the slicing itself).

MODE selects the matmul numerics:
  fp32  - exact fp32 matmuls (4 PE cycles/row)
  f32r  - fp32 storage, replicated-mode matmul (1 cycle/row, reduced
          multiply precision)
  split - weights & x shipped as bf16 hi+lo pairs; x@W = xhi@Whi +
          xlo@Whi + xhi@Wlo accumulated in fp32 PSUM (2 cycles/row,
          ~1e-5 rel err, full fp32 DMA traffic)
  bf16  - plain bf16 weights/x (1 cycle/row, half DMA traffic,
          ~2e-3 rel err)
"""

import math
import os

import numpy as np

B = 8
E = 4096
H = 32
D = 128
L = 4095
N_CORES = 8
HPC = H // N_CORES  # heads per core = 4
F = HPC * D  # per-core head width = 512
ET = E // 128  # e tiles = 32
SCALE = 1.0 / math.sqrt(D)
BIAS = -math.log(L)

MODE = os.environ.get("MHA_MODE", "split")

# DMA chunking for the q/k/v weight streams: e-tiles per DMA (1 MiB)
EC = 4
NCHUNK = ET // EC

_CACHE = {}


def _build_program(mode):
    import concourse.mybir as mybir
    import concourse.tile as tile
    from concourse import bacc
    from concourse.masks import make_identity

    fp32 = mybir.dt.float32
    bf16 = mybir.dt.bfloat16
    split = mode == "split"
    wdt = bf16 if mode in ("split", "bf16") else fp32
    if mode == "f32r":
        wdt = mybir.dt.float32r
    # dtype used on the ovec->zT transpose path (tsrc/ident/psum)
    tdt = bf16 if mode in ("split", "bf16") else fp32
    # stationary x holds hi (rows 0-7) and lo (rows 32-39) halves in
    # split mode; partition-base APs must start at multiples of 32
    XW = 40 if split else B

    nc = bacc.Bacc("TRN2", target_bir_lowering=False)

    # split: q/k/v ship chunk-interleaved hi/lo rows ([2E, F]); wo ships
    # stacked halves ([2, F, E]) whose strides merge naturally
    qsh = [2 * E, F] if split else [E, F]
    osh = [2, F, E] if split else [F, E]
    wq = nc.dram_tensor("wq_t", qsh, wdt, kind="ExternalInput").ap()
    wk = nc.dram_tensor("wk_t", qsh, wdt, kind="ExternalInput").ap()
    wv = nc.dram_tensor("wv_t", qsh, wdt, kind="ExternalInput").ap()
    wo = nc.dram_tensor("wo_t", osh, wdt, kind="ExternalInput").ap()
    xw_in = 2 * B if split else B
    xt = nc.dram_tensor("xt", [E, xw_in], wdt, kind="ExternalInput").ap()
    out = nc.dram_tensor("out_p", [B, E], fp32, kind="ExternalOutput").ap()

    if split:
        def wr(ap):  # [2E, F] chunk-interleaved -> [128, NCHUNK, 2, EC, F]
            return ap.rearrange("(c s j p) f -> p c s j f", p=128, s=2, j=EC)

        wo_r = wo.rearrange("s (t p) e -> p s t e", p=128)
    else:
        def wr(ap):  # [E, F] -> [128, ET, F]
            return ap.rearrange("(c p) f -> p c f", p=128)

        wo_r = wo.rearrange("(t p) e -> p t e", p=128)  # [128, HPC, E]
    xt_r = xt.rearrange("(t p) b -> p t b", p=128)  # [128, ET, xw_in]

    with tile.TileContext(nc) as tc:
        with (
            tc.tile_pool(name="const", bufs=1) as const_pool,
            tc.tile_pool(name="wqp", bufs=3) as wq_pool,
            tc.tile_pool(name="wkp", bufs=3) as wk_pool,
            tc.tile_pool(name="wvp", bufs=3) as wv_pool,
            tc.tile_pool(name="wop", bufs=6) as wo_pool,
            tc.tile_pool(name="small", bufs=1) as small_pool,
            tc.tile_pool(name="outp", bufs=3) as out_pool,
            tc.tile_pool(name="ps_qkv", bufs=1, space="PSUM") as ps_qkv,
            tc.tile_pool(name="ps_t", bufs=1, space="PSUM") as ps_t,
            tc.tile_pool(name="ps_o", bufs=4, space="PSUM") as ps_o,
        ):
            ident = const_pool.tile([128, 128], tdt)
            make_identity(nc, ident[:])
            bias_sb = const_pool.tile([128, 1], fp32, tag="bias")
            nc.gpsimd.memset(bias_sb[:], BIAS)

            xin_sb = const_pool.tile([128, ET, xw_in], wdt, tag="xin")
            nc.scalar.dma_start(xin_sb[:], xt_r)
            if split:
                # widen to [xhi | 0 | xlo] (cols 0-7 / 32-39) and build the
                # [xhi | 0] stationary for the lo-weight pass on device
                xt_sb = const_pool.tile([128, ET, XW], wdt, tag="xt40")
                nc.gpsimd.memset(xt_sb[:], 0.0)
                nc.vector.tensor_copy(xt_sb[:, :, :B], xin_sb[:, :, :B])
                nc.vector.tensor_copy(xt_sb[:, :, 32:40], xin_sb[:, :, B:])
                xt0_sb = const_pool.tile([128, ET, XW], wdt, tag="xt0")
                nc.gpsimd.memset(xt0_sb[:], 0.0)
                nc.vector.tensor_copy(xt0_sb[:, :, :B], xin_sb[:, :, :B])
            else:
                xt_sb = xin_sb

            # ---- q/k/v projections ----
            psum_q = ps_qkv.tile([XW, F], fp32, tag="psq")
            psum_k = ps_qkv.tile([XW, F], fp32, tag="psk")
            psum_v = ps_qkv.tile([XW, F], fp32, tag="psv")

            for c in range(NCHUNK):
                sls = slice(c * EC, (c + 1) * EC)
                if split:
                    w_sb = {}
                    for nm, ap, pool in (
                        ("q", wr(wq), wq_pool),
                        ("k", wr(wk), wk_pool),
                        ("v", wr(wv), wv_pool),
                    ):
                        t2 = pool.tile([128, 2, EC, F], wdt, tag="w" + nm)
                        nc.sync.dma_start(t2[:], ap[:, c])
                        w_sb[nm] = t2
                else:
                    w_sb = {}
                    for nm, ap, pool in (
                        ("q", wr(wq), wq_pool),
                        ("k", wr(wk), wk_pool),
                        ("v", wr(wv), wv_pool),
                    ):
                        t1 = pool.tile([128, EC, F], wdt, tag="w" + nm)
                        nc.sync.dma_start(t1[:], ap[:, sls, :])
                        w_sb[nm] = t1
                for j in range(EC):
                    t = c * EC + j
                    first, last = t == 0, t == ET - 1
                    lhs_full = xt_sb[:, t, :]
                    for nm, ps in (("q", psum_q), ("k", psum_k), ("v", psum_v)):
                        if split:
                            nc.tensor.matmul(
                                ps[:], lhs_full, w_sb[nm][:, 0, j, :],
                                start=first, stop=False,
                            )
                            nc.tensor.matmul(
                                ps[:], xt0_sb[:, t, :], w_sb[nm][:, 1, j, :],
                                start=False, stop=last,
                            )
                        else:
                            nc.tensor.matmul(
                                ps[:], lhs_full, w_sb[nm][:, j, :],
                                start=first, stop=last,
                            )

            # ---- combine split halves; closed-form attention ----
            q_sb = small_pool.tile([B, F], fp32, tag="qsb")
            k_sb = small_pool.tile([B, F], fp32, tag="ksb")
            v_sb = small_pool.tile([B, F], fp32, tag="vsb")
            if split:
                tmp = small_pool.tile([B, F], fp32, tag="tmp")
                for ps, dst in ((psum_q, q_sb), (psum_k, k_sb), (psum_v, v_sb)):
                    nc.vector.tensor_copy(tmp[:], ps[32:40, :])
                    nc.vector.tensor_tensor(
                        dst[:], ps[:B, :], tmp[:], mybir.AluOpType.add
                    )
            else:
                nc.vector.tensor_copy(q_sb[:], psum_q[:])
                nc.vector.tensor_copy(k_sb[:], psum_k[:])
                nc.vector.tensor_copy(v_sb[:], psum_v[:])

            s0 = small_pool.tile([B, HPC], fp32, tag="s0")
            s1 = small_pool.tile([B, HPC], fp32, tag="s1")
            qk = small_pool.tile([B, F], fp32, tag="qk")
            tt = small_pool.tile([B, HPC], fp32, tag="tt")
            p = small_pool.tile([B, HPC], fp32, tag="p")
            ovec = small_pool.tile([B, F], fp32, tag="ovec")

            nc.vector.reduce_sum(
                s0[:], q_sb[:].rearrange("b (h d) -> b h d", d=D),
                axis=mybir.AxisListType.X,
            )
            nc.vector.tensor_tensor(
                qk[:], q_sb[:], k_sb[:], mybir.AluOpType.mult
            )
            nc.vector.reduce_sum(
                s1[:], qk[:].rearrange("b (h d) -> b h d", d=D),
                axis=mybir.AxisListType.X,
            )
            nc.vector.tensor_tensor(
                tt[:], s1[:], s0[:], mybir.AluOpType.subtract
            )
            # p = sigmoid((s1 - s0) * (1/sqrt(D)) - ln(L))
            nc.scalar.activation(
                p[:], tt[:], mybir.ActivationFunctionType.Sigmoid,
                bias=bias_sb[:B, :], scale=SCALE,
            )
            vm1 = small_pool.tile([B, F], fp32, tag="vm1")
            nc.vector.tensor_scalar_add(vm1[:], v_sb[:], -1.0)
            for h in range(HPC):
                sl = slice(h * D, (h + 1) * D)
                nc.vector.tensor_scalar(
                    ovec[:, sl], vm1[:, sl],
                    p[:, h : h + 1], 1.0,
                    mybir.AluOpType.mult, mybir.AluOpType.add,
                )

            # ---- transpose ovec -> zT [f, b] (PE transposes) ----
            if split:
                # hi/lo split of ovec: hi rows 0-7, lo rows 32-39
                z2 = small_pool.tile([XW, F], tdt, tag="z2")
                zf = small_pool.tile([B, F], fp32, tag="zf")
                nc.gpsimd.memset(z2[:], 0.0)
                nc.vector.tensor_copy(z2[:B, :], ovec[:])  # cast to bf16
                nc.vector.tensor_copy(zf[:], z2[:B, :])  # back to f32
                nc.vector.tensor_tensor(
                    zf[:], ovec[:], zf[:], mybir.AluOpType.subtract
                )
                nc.vector.tensor_copy(z2[32:40, :], zf[:])  # lo in bf16
                z3 = small_pool.tile([XW, F], tdt, tag="z3")
                nc.gpsimd.memset(z3[:], 0.0)
                nc.vector.tensor_copy(z3[:B, :], z2[:B, :])
                tsrc, tp = z2, XW
            elif mode == "bf16":
                z1 = small_pool.tile([B, F], tdt, tag="z1")
                nc.vector.tensor_copy(z1[:], ovec[:])
                tsrc, tp = z1, B
            else:
                tsrc, tp = ovec, B
            zt_sb = small_pool.tile([128, HPC, XW], wdt, tag="zt")
            for t in range(HPC):
                zt_ps = ps_t.tile([128, tp], tdt, tag="ztps")
                nc.tensor.transpose(
                    zt_ps[:], tsrc[:, t * 128 : (t + 1) * 128], ident[:tp, :tp]
                )
                nc.vector.tensor_copy(zt_sb[:, t, :], zt_ps[:])
            if split:
                zt0_sb = small_pool.tile([128, HPC, XW], wdt, tag="zt0")
                for t in range(HPC):
                    zt_ps = ps_t.tile([128, tp], tdt, tag="ztps")
                    nc.tensor.transpose(
                        zt_ps[:], z3[:, t * 128 : (t + 1) * 128], ident[:tp, :tp]
                    )
                    nc.vector.tensor_copy(zt0_sb[:, t, :], zt_ps[:])

            # ---- out-proj (wo streamed in per-chunk DMAs so the MMs
            # pipeline with the transfer instead of a serial tail) ----
            # 6 x 512-wide chunks, then 4 x 256-wide for a shorter drain
            chunks = [(k * 512, 512) for k in range(6)] + [
                (3072 + k * 256, 256) for k in range(4)
            ]
            NOC = len(chunks)
            o_acc = small_pool.tile([B, E], fp32, tag="oacc")
            for c2, (off, w) in enumerate(chunks):
                sl2 = slice(off, off + w)
                if split:
                    wo_sb = wo_pool.tile([128, 2, HPC, 512], wdt, tag="wo")
                    nc.sync.dma_start(wo_sb[:, :, :, :w], wo_r[:, :, :, sl2])
                else:
                    wo_sb = wo_pool.tile([128, HPC, 512], wdt, tag="wo")
                    nc.sync.dma_start(wo_sb[:, :, :w], wo_r[:, :, sl2])
                psum_o = ps_o.tile([XW, 512], fp32, tag="pso")
                psum_o = psum_o[:, :w]
                for t in range(HPC):
                    if split:
                        nc.tensor.matmul(
                            psum_o[:], zt_sb[:, t, :], wo_sb[:, 0, t, :w],
                            start=(t == 0), stop=False,
                        )
                        nc.tensor.matmul(
                            psum_o[:], zt0_sb[:, t, :], wo_sb[:, 1, t, :w],
                            start=False, stop=(t == HPC - 1),
                        )
                    else:
                        nc.tensor.matmul(
                            psum_o[:], zt_sb[:, t, :], wo_sb[:, t, :w],
                            start=(t == 0), stop=(t == HPC - 1),
                        )
                if split:
                    ol_sb = out_pool.tile([B, 512], fp32, tag="olsb")
                    nc.vector.tensor_copy(ol_sb[:, :w], psum_o[32:40, :])
                    nc.vector.tensor_tensor(
                        o_acc[:, sl2], psum_o[:B, :], ol_sb[:, :w],
                        mybir.AluOpType.add,
                    )
                else:
                    nc.vector.tensor_copy(o_acc[:, sl2], psum_o[:])
                if c2 == 5:
                    # staggered early writes keep all write dispatch (and
                    # its HWDGE setup) off the final-write critical path
                    nc.scalar.dma_start(out[:, :3072], o_acc[:, :3072])
                elif c2 == NOC - 2:
                    nc.scalar.dma_start(out[:, 3072:3840], o_acc[:, 3072:3840])
            nc.sync.dma_start(out[:, 3840:], o_acc[:, 3840:])

    nc.compile()
    return nc


def _get_program(mode=MODE):
    key = "nc_" + mode
    if key not in _CACHE:
        _CACHE[key] = _build_program(mode)
    return _CACHE[key]


def _split_pair(a):
    import ml_dtypes

    hi = a.astype(ml_dtypes.bfloat16)
    lo = (a - hi.astype(np.float32)).astype(ml_dtypes.bfloat16)
    return hi, lo


def _shard_inputs(x, Wq, Wk, Wv, Wo, mode=MODE):
    import ml_dtypes

    xt = np.ascontiguousarray(x.reshape(B, E).T)
    in_maps = []
    if mode == "split":
        xh, xl = _split_pair(xt)
        xt2 = np.concatenate([xh, xl], axis=1)  # [E, 16]
    elif mode == "bf16":
        xt2 = xt.astype(ml_dtypes.bfloat16)
    else:
        xt2 = xt
    for c in range(N_CORES):
        rows = slice(c * F, (c + 1) * F)
        wqt = np.ascontiguousarray(Wq[rows, :].T)
        wkt = np.ascontiguousarray(Wk[rows, :].T)
        wvt = np.ascontiguousarray(Wv[rows, :].T)
        wot = np.ascontiguousarray(Wo[:, rows].T)
        m = {"xt": xt2}
        if mode == "split":
            for nm, w in (("wq", wqt), ("wk", wkt), ("wv", wvt)):
                hi, lo = _split_pair(w)
                hi = hi.reshape(NCHUNK, EC * 128, F)
                lo = lo.reshape(NCHUNK, EC * 128, F)
                m[nm + "_t"] = np.ascontiguousarray(
                    np.stack([hi, lo], axis=1)
                ).reshape(2 * E, F)
            hi, lo = _split_pair(wot)
            m["wo_t"] = np.stack([hi, lo])
        elif mode == "bf16":
            for nm, w in (("wq", wqt), ("wk", wkt), ("wv", wvt), ("wo", wot)):
                m[nm + "_t"] = w.astype(ml_dtypes.bfloat16)
        else:
            m.update(wq_t=wqt, wk_t=wkt, wv_t=wvt, wo_t=wot)
        in_maps.append(m)
    return in_maps


def kernel(x, Wq, Wk, Wv, Wo, _trace=False, **_unused):
    from concourse.bass_utils import run_bass_kernel_spmd

    nc = _get_program()
    in_maps = _shard_inputs(
        np.asarray(x, dtype=np.float32),
        np.asarray(Wq, dtype=np.float32),
        np.asarray(Wk, dtype=np.float32),
        np.asarray(Wv, dtype=np.float32),
        np.asarray(Wo, dtype=np.float32),
    )
    core_ids = list(range(N_CORES))

    def _run(trace):
        return run_bass_kernel_spmd(nc, in_maps, core_ids, trace=trace)

    res = None
    if _trace:
        try:
            res = _run(True)
        except Exception:
            # NTFF profiling hooks unavailable in this environment
            res = None
    if res is None:
        # transient device wedges (NRT_EXEC_UNIT_UNRECOVERABLE) heal after
        # a terminal-side reset; tear down the PJRT client and back off
        # before each retry
        import time as _time

        last = None
        for attempt in range(3):
            try:
                res = _run(False)
                break
            except Exception as e:
                last = e
                try:
                    import jax._src.xla_bridge as _xb

                    _xb._clear_backends()
                except Exception:
                    pass
                _time.sleep(15 * (attempt + 1))
        else:
            raise last
    _CACHE["last_results"] = res
    acc = np.zeros((B, E), np.float32)
    for r in res.results:
        acc += r["out_p"]
    return acc.reshape(B, 1, E)



# revision 6
# speedup vs baseline: 3.0081x; 3.0081x over previous
"""Trainium2 Bass kernel for nn_MultiHeadAttention_69930657513858.

Single-token (decode) multi-head attention, B=8, E=4096, H=32 heads of
D=128, with a KV cache that is identically ones (length L=4095).

Because the cache is all-ones, attention collapses to a closed form:
  scores = [s0]*L ++ [s1],  s0 = sum_d(q)/sqrt(D), s1 = (q.k)/sqrt(D)
  softmax => p_last = sigmoid(s1 - s0 - ln(L)); cache mass = 1 - p_last
  o = 1 + p_last*(v - 1)
and since s1 - s0 = sum_d q*(k-1), the whole attention needs one
partition-dim reduction.  Furthermore out = o @ Wo^T splits into
rowsum(Wo) (computed exactly on the host, batch-independent) plus
Wo @ delta with delta = p*(v-1), so all device GEMM traffic tolerates
low precision: every weight ships as fp8 e4m3 (scaled 2^10), which is
4x less HBM/DMA traffic than fp32 -- the sole bottleneck of this
memory-bound decode step.  x and delta are split into fp8 hi+lo pairs
(residual splitting) so their quantization is negligible; measured
output rel err ~5e-3 vs the 2e-2 gate.

Matmuls keep the WEIGHT stationary and stream the tiny activations
(moving free dim = 8/16), so PE time is ~2us against ~24us of DMA.
Layouts are chosen so the head dim d lands on partitions: q^T/k^T/v^T
tiles are [128d, 4h, 8b], making s=sum_d a ones-vector matmul and the
per-(h,b) sigmoid a [1,32] op; p broadcasts back over partitions with a
rank-1 matmul against a constant-64 row (folding the delta fp8 scale).

Sharding: tensor-parallel over heads, 4 heads per core (Wq/Wk/Wv row
slices, Wo column slices); per-core out-proj partials are summed on the
host together with rowsum(Wo) (the "all-reduce").
"""

import math
import os

import numpy as np

B = 8
E = 4096
H = 32
D = 128
L = 4095
N_CORES = 8
HPC = H // N_CORES  # heads per core = 4
F = HPC * D  # per-core head width = 512
ET = E // 128  # e tiles = 32
SCALE = 1.0 / math.sqrt(D)
BIAS = -math.log(L)

WS = 1024.0  # weight fp8 scale (2^10)
XS = 16.0  # x fp8 scale (2^4)
DS = 64.0  # delta fp8 scale (2^6), folded into the p broadcast
QSC = WS * XS  # q/k/v psum scale (2^14)
OSC = WS * DS  # out psum scale (2^16)

MODE = os.environ.get("MHA_MODE", "fp8")

NWOC = 8  # wo e-chunks (512 cols each)

_CACHE = {}


def _build_program():
    import concourse.mybir as mybir
    import concourse.tile as tile
    from concourse import bacc

    fp32 = mybir.dt.float32
    f8 = mybir.dt.float8e4

    nc = bacc.Bacc("TRN2", target_bir_lowering=False)

    # all HBM operands are packed partition-major on the host so every
    # DMA descriptor is a contiguous >=512B run (full 360GB/s in one shot)
    xt = nc.dram_tensor("xt", [128, ET * 2 * B], f8, kind="ExternalInput").ap()
    wq = nc.dram_tensor("wq_t", [128, ET * F], f8, kind="ExternalInput").ap()
    wk = nc.dram_tensor("wk_t", [128, ET * F], f8, kind="ExternalInput").ap()
    wv = nc.dram_tensor("wv_t", [128, ET * F], f8, kind="ExternalInput").ap()
    wo = nc.dram_tensor("wo_t", [128, HPC * E], f8, kind="ExternalInput").ap()
    out = nc.dram_tensor("out_p", [128, ET * B], fp32, kind="ExternalOutput").ap()

    xt_r = xt.rearrange("p (t s) -> p t s", t=ET)  # [128, 32, 16]
    wq_r = wq.rearrange("p (t f) -> p t f", t=ET)  # [128, 32, 512]
    wk_r = wk.rearrange("p (t f) -> p t f", t=ET)
    wv_r = wv.rearrange("p (t f) -> p t f", t=ET)
    wo_r = wo.rearrange("p (t e) -> p t e", t=HPC)  # [128, 4, 4096]
    out_r = out.rearrange("p (t b) -> p t b", t=ET)  # [128, 32, 8]

    with tile.TileContext(nc) as tc:
        with (
            tc.tile_pool(name="const", bufs=1) as const_pool,
            tc.tile_pool(name="wqkv", bufs=3) as w_pool,
            tc.tile_pool(name="wop", bufs=NWOC) as wo_pool,
            tc.tile_pool(name="small", bufs=1) as small_pool,
            tc.tile_pool(name="ps_q", bufs=1, space="PSUM") as ps_q,
            tc.tile_pool(name="ps_k", bufs=1, space="PSUM") as ps_k,
            tc.tile_pool(name="ps_v", bufs=1, space="PSUM") as ps_v,
            tc.tile_pool(name="ps_s", bufs=1, space="PSUM") as ps_s,
            tc.tile_pool(name="ps_o", bufs=1, space="PSUM") as ps_o,
        ):
            ones_sb = const_pool.tile([128, 1], fp32, tag="ones")
            nc.gpsimd.memset(ones_sb[:], 1.0)
            c64_sb = const_pool.tile([1, 128], fp32, tag="c64")
            nc.gpsimd.memset(c64_sb[:], DS)
            neg1_sb = const_pool.tile([128, 1], fp32, tag="neg1")
            nc.gpsimd.memset(neg1_sb[:], -1.0)
            bias_sb = const_pool.tile([1, 1], fp32, tag="bias")
            nc.gpsimd.memset(bias_sb[:], BIAS)

            # ---- input DMAs (SP queue, transfers serialize on the DMA
            # engines in this order; weights qkv first, wo last) ----
            x_sb = const_pool.tile([128, ET, 2 * B], f8, tag="x")
            nc.sync.dma_start(x_sb[:], xt_r)
            wq_sb = w_pool.tile([128, ET, F], f8, tag="wq")
            nc.sync.dma_start(wq_sb[:], wq_r)
            wk_sb = w_pool.tile([128, ET, F], f8, tag="wk")
            nc.sync.dma_start(wk_sb[:], wk_r)
            wv_sb = w_pool.tile([128, ET, F], f8, tag="wv")
            nc.sync.dma_start(wv_sb[:], wv_r)
            wo_sb = []
            for c in range(NWOC):
                t = wo_pool.tile([128, HPC, 512], f8, tag="wo")
                nc.sync.dma_start(t[:], wo_r[:, :, c * 512 : (c + 1) * 512])
                wo_sb.append(t)

            # ---- q/k/v projections: weight stationary, x moving ----
            # psum [128d, 4h*8b], accumulated over 32 e-tiles x (hi, lo)
            psq = ps_q.tile([128, HPC * B], fp32, tag="psq")
            psk = ps_k.tile([128, HPC * B], fp32, tag="psk")
            psv = ps_v.tile([128, HPC * B], fp32, tag="psv")
            for w_sb, ps in ((wq_sb, psq), (wk_sb, psk), (wv_sb, psv)):
                for ft in range(HPC):
                    dst = ps[:, ft * B : (ft + 1) * B]
                    for et in range(ET):
                        lhs = w_sb[:, et, ft * 128 : (ft + 1) * 128]
                        nc.tensor.matmul(
                            dst, lhs, x_sb[:, et, :B],
                            start=(et == 0), stop=False,
                        )
                        nc.tensor.matmul(
                            dst, lhs, x_sb[:, et, B:],
                            start=False, stop=(et == ET - 1),
                        )

            # ---- closed-form attention (scale QSC on q/k/v psums) ----
            km1 = small_pool.tile([128, HPC * B], fp32, tag="km1")
            nc.scalar.activation(
                km1[:], psk[:], mybir.ActivationFunctionType.Identity,
                bias=neg1_sb[:], scale=1.0 / QSC,
            )  # k - 1, exact scale
            vm1 = small_pool.tile([128, HPC * B], fp32, tag="vm1")
            nc.scalar.activation(
                vm1[:], psv[:], mybir.ActivationFunctionType.Identity,
                bias=neg1_sb[:], scale=1.0 / QSC,
            )  # v - 1
            qkm = small_pool.tile([128, HPC * B], fp32, tag="qkm")
            nc.vector.tensor_tensor(
                qkm[:], psq[:], km1[:], mybir.AluOpType.mult
            )  # q*(k-1), scale QSC
            # tt[1, 32] = sum_d q*(k-1) = s1 - s0 (scale QSC)
            ps_tt = ps_s.tile([1, HPC * B], fp32, tag="pstt")
            nc.tensor.matmul(ps_tt[:], ones_sb[:], qkm[:], start=True, stop=True)
            p_sb = small_pool.tile([1, HPC * B], fp32, tag="p")
            nc.scalar.activation(
                p_sb[:], ps_tt[:], mybir.ActivationFunctionType.Sigmoid,
                bias=bias_sb[:], scale=SCALE / QSC,
            )
            # broadcast p over partitions, folding the delta fp8 scale:
            # pb[128, 32] = p * DS
            ps_pb = ps_s.tile([128, HPC * B], fp32, tag="pspb")
            nc.tensor.matmul(ps_pb[:], c64_sb[:], p_sb[:], start=True, stop=True)
            dsc = small_pool.tile([128, HPC * B], fp32, tag="dsc")
            nc.vector.tensor_tensor(
                dsc[:], vm1[:], ps_pb[:], mybir.AluOpType.mult
            )  # delta * DS
            dhi = small_pool.tile([128, HPC * B], f8, tag="dhi")
            nc.vector.tensor_copy(dhi[:], dsc[:])
            dhf = small_pool.tile([128, HPC * B], fp32, tag="dhf")
            nc.vector.tensor_copy(dhf[:], dhi[:])
            dlo = small_pool.tile([128, HPC * B], f8, tag="dlo")
            nc.vector.tensor_tensor(
                dlo[:], dsc[:], dhf[:], mybir.AluOpType.subtract
            )

            # ---- out-proj: wo stationary, delta hi/lo moving; psum is
            # out^T [128e, 8b] per e-tile, scale OSC ----
            pso = ps_o.tile([128, ET, B], fp32, tag="pso")
            out_sb = const_pool.tile([128, ET, B], fp32, tag="osb")
            for c in range(NWOC):
                for el in range(4):
                    et = c * 4 + el
                    dst = pso[:, et, :]
                    for ft in range(HPC):
                        lhs = wo_sb[c][:, ft, el * 128 : (el + 1) * 128]
                        nc.tensor.matmul(
                            dst, lhs, dhi[:, ft * B : (ft + 1) * B],
                            start=(ft == 0), stop=False,
                        )
                        nc.tensor.matmul(
                            dst, lhs, dlo[:, ft * B : (ft + 1) * B],
                            start=False, stop=(ft == HPC - 1),
                        )
                sl = slice(c * 4, (c + 1) * 4)
                nc.scalar.activation(
                    out_sb[:, sl, :], pso[:, sl, :],
                    mybir.ActivationFunctionType.Copy, scale=1.0 / OSC,
                )
                # staggered writes keep dispatch off the final critical path
                if c == NWOC - 3:
                    nc.scalar.dma_start(
                        out_r[:, : 4 * (c + 1)], out_sb[:, : 4 * (c + 1), :]
                    )
                elif c == NWOC - 2:
                    nc.scalar.dma_start(
                        out_r[:, 4 * c : 4 * (c + 1)], out_sb[:, sl, :]
                    )
            nc.scalar.dma_start(out_r[:, 4 * (NWOC - 1) :], out_sb[:, 4 * (NWOC - 1) :, :])

    nc.compile()
    return nc


def _get_program(mode=MODE):
    key = "nc_" + mode
    if key not in _CACHE:
        _CACHE[key] = _build_program()
    return _CACHE[key]


def _pack_pmajor(a, tiles):
    """[tiles*128, w] -> [128, tiles*w] partition-major contiguous."""
    w = a.shape[1]
    return np.ascontiguousarray(
        a.reshape(tiles, 128, w).transpose(1, 0, 2).reshape(128, tiles * w)
    )


def _shard_inputs(x, Wq, Wk, Wv, Wo, mode=MODE):
    import ml_dtypes

    f8 = ml_dtypes.float8_e4m3

    def q8(a):
        return np.clip(a, -240.0, 240.0).astype(f8)

    xt = x.reshape(B, E).T * XS  # [E, 8]
    xh = q8(xt)
    xl = q8(xt - xh.astype(np.float32))
    x2 = _pack_pmajor(np.concatenate([xh, xl], axis=1), ET)  # [128, 512]

    in_maps = []
    for c in range(N_CORES):
        rows = slice(c * F, (c + 1) * F)
        m = {
            "xt": x2,
            "wq_t": _pack_pmajor(q8(Wq[rows, :].T * WS), ET),
            "wk_t": _pack_pmajor(q8(Wk[rows, :].T * WS), ET),
            "wv_t": _pack_pmajor(q8(Wv[rows, :].T * WS), ET),
            "wo_t": _pack_pmajor(q8(Wo[:, rows].T * WS), HPC),
        }
        in_maps.append(m)
    return in_maps


def kernel(x, Wq, Wk, Wv, Wo, _trace=False, **_unused):
    from concourse.bass_utils import run_bass_kernel_spmd

    x = np.asarray(x, dtype=np.float32)
    Wq = np.asarray(Wq, dtype=np.float32)
    Wk = np.asarray(Wk, dtype=np.float32)
    Wv = np.asarray(Wv, dtype=np.float32)
    Wo = np.asarray(Wo, dtype=np.float32)

    nc = _get_program()
    in_maps = _shard_inputs(x, Wq, Wk, Wv, Wo)
    core_ids = list(range(N_CORES))

    def _run(trace):
        return run_bass_kernel_spmd(nc, in_maps, core_ids, trace=trace)

    res = None
    if _trace:
        try:
            res = _run(True)
        except Exception:
            # NTFF profiling hooks unavailable in this environment
            res = None
    if res is None:
        # transient device wedges (NRT_EXEC_UNIT_UNRECOVERABLE) heal after
        # a terminal-side reset; tear down the PJRT client and back off
        # before each retry
        import time as _time

        last = None
        for attempt in range(3):
            try:
                res = _run(False)
                break
            except Exception as e:
                last = e
                try:
                    import jax._src.xla_bridge as _xb

                    _xb._clear_backends()
                except Exception:
                    pass
                _time.sleep(15 * (attempt + 1))
        else:
            raise last
    _CACHE["last_results"] = res

    acc = np.zeros((B, E), np.float32)
    for r in res.results:
        # out_p[p, et, b] -> partial[b, et*128 + p]
        buf = np.asarray(r["out_p"], np.float32).reshape(128, ET, B)
        acc += buf.transpose(2, 1, 0).reshape(B, E)
    acc += Wo.sum(axis=1, dtype=np.float32)[None, :]
    return acc.reshape(B, 1, E)


# revision 7
# speedup vs baseline: 3.3797x; 1.1235x over previous
"""Trainium2 Bass kernel for nn_MultiHeadAttention_69930657513858.

Single-token (decode) multi-head attention, B=8, E=4096, H=32 heads of
D=128, with a KV cache that is identically ones (length L=4095).

Because the cache is all-ones, attention collapses to a closed form:
  scores = [s0]*L ++ [s1],  s0 = sum_d(q)/sqrt(D), s1 = (q.k)/sqrt(D)
  softmax => p_last = sigmoid(s1 - s0 - ln(L)); cache mass = 1 - p_last
  o = 1 + p_last*(v - 1)
and since s1 - s0 = sum_d q*(k-1), the whole attention needs one
partition-dim reduction.  Furthermore out = o @ Wo^T splits into
rowsum(Wo) (computed exactly on the host, batch-independent) plus
Wo @ delta with delta = p*(v-1), so all device GEMM traffic tolerates
low precision: every weight ships as fp8 e4m3 (scaled 2^10), which is
4x less HBM/DMA traffic than fp32 -- the sole bottleneck of this
memory-bound decode step.  x and delta are split into fp8 hi+lo pairs
(residual splitting) so their quantization is negligible; measured
output rel err ~5e-3 vs the 2e-2 gate.

Matmuls keep the WEIGHT stationary and stream the tiny activations
(moving free dim = 8/16), so PE time is ~2us against ~24us of DMA.
Layouts are chosen so the head dim d lands on partitions: q^T/k^T/v^T
tiles are [128d, 4h, 8b], making s=sum_d a ones-vector matmul and the
per-(h,b) sigmoid a [1,32] op; p broadcasts back over partitions with a
rank-1 matmul against a constant-64 row (folding the delta fp8 scale).

Sharding: tensor-parallel over heads, 4 heads per core (Wq/Wk/Wv row
slices, Wo column slices); per-core out-proj partials are summed on the
host together with rowsum(Wo) (the "all-reduce").
"""

import math
import os

import numpy as np

B = 8
E = 4096
H = 32
D = 128
L = 4095
N_CORES = 8
HPC = H // N_CORES  # heads per core = 4
F = HPC * D  # per-core head width = 512
ET = E // 128  # e tiles = 32
SCALE = 1.0 / math.sqrt(D)
BIAS = -math.log(L)

WS = 1024.0  # weight fp8 scale (2^10)
XS = 16.0  # x fp8 scale (2^4)
DS = 64.0  # delta fp8 scale (2^6), folded into the p broadcast
QSC = WS * XS  # q/k/v psum scale (2^14)
OSC = WS * DS  # out psum scale (2^16)

MODE = os.environ.get("MHA_MODE", "fp8")

NWOC = 8  # wo e-chunks (512 cols each)

_CACHE = {}


def _build_program():
    import concourse.mybir as mybir
    import concourse.tile as tile
    from concourse import bacc

    fp32 = mybir.dt.float32
    f8 = mybir.dt.float8e4

    nc = bacc.Bacc("TRN2", target_bir_lowering=False)

    # all HBM operands are packed partition-major on the host so every
    # DMA descriptor is a contiguous >=512B run (full 360GB/s in one shot)
    xt = nc.dram_tensor("xt", [128, ET * 2 * B], f8, kind="ExternalInput").ap()
    wq = nc.dram_tensor("wq_t", [128, ET * F], f8, kind="ExternalInput").ap()
    wk = nc.dram_tensor("wk_t", [128, ET * F], f8, kind="ExternalInput").ap()
    wv = nc.dram_tensor("wv_t", [128, ET * F], f8, kind="ExternalInput").ap()
    wo = nc.dram_tensor("wo_t", [128, HPC * E], f8, kind="ExternalInput").ap()
    out = nc.dram_tensor("out_p", [128, ET * B], fp32, kind="ExternalOutput").ap()

    xt_r = xt.rearrange("p (t s) -> p t s", t=ET)  # [128, 32, 16]
    wq_r = wq.rearrange("p (t f) -> p t f", t=ET)  # [128, 32, 512]
    wk_r = wk.rearrange("p (t f) -> p t f", t=ET)
    wv_r = wv.rearrange("p (t f) -> p t f", t=ET)
    wo_r = wo.rearrange("p (t e) -> p t e", t=HPC)  # [128, 4, 4096]
    out_r = out.rearrange("p (t b) -> p t b", t=ET)  # [128, 32, 8]

    with tile.TileContext(nc) as tc:
        with (
            tc.tile_pool(name="const", bufs=1) as const_pool,
            tc.tile_pool(name="wqkv", bufs=3) as w_pool,
            tc.tile_pool(name="wop", bufs=NWOC) as wo_pool,
            tc.tile_pool(name="small", bufs=1) as small_pool,
            tc.tile_pool(name="ps_q", bufs=1, space="PSUM") as ps_q,
            tc.tile_pool(name="ps_k", bufs=1, space="PSUM") as ps_k,
            tc.tile_pool(name="ps_v", bufs=1, space="PSUM") as ps_v,
            tc.tile_pool(name="ps_s", bufs=1, space="PSUM") as ps_s,
            tc.tile_pool(name="ps_o", bufs=3, space="PSUM") as ps_o,
        ):
            ones_sb = const_pool.tile([128, 1], fp32, tag="ones")
            nc.gpsimd.memset(ones_sb[:], 1.0)
            c64_sb = const_pool.tile([1, 128], fp32, tag="c64")
            nc.gpsimd.memset(c64_sb[:], DS)
            bias_sb = const_pool.tile([1, 1], fp32, tag="bias")
            nc.gpsimd.memset(bias_sb[:], BIAS)
            # warm the Sigmoid activation table off the critical path (the
            # cost model charges a 1.28us table load at first use)
            scr_sb = const_pool.tile([1, 1], fp32, tag="scr")
            nc.scalar.activation(
                scr_sb[:], bias_sb[:], mybir.ActivationFunctionType.Sigmoid,
                bias=bias_sb[:], scale=1.0,
            )

            # ---- input DMAs (SP queue, transfers serialize on the DMA
            # engines in this order; wv last of q/k/v: its post-arrival
            # chain (vm1 -> delta) is the shortest, wo chunks last) ----
            x_sb = const_pool.tile([128, ET, 2 * B], f8, tag="x")
            nc.sync.dma_start(x_sb[:], xt_r)
            wq_sb = w_pool.tile([128, ET, F], f8, tag="wq")
            nc.sync.dma_start(wq_sb[:], wq_r)
            wk_sb = w_pool.tile([128, ET, F], f8, tag="wk")
            nc.sync.dma_start(wk_sb[:], wk_r)
            wv_sb = w_pool.tile([128, ET, F], f8, tag="wv")
            nc.sync.dma_start(wv_sb[:], wv_r)
            wo_sb = []
            for c in range(NWOC):
                t = wo_pool.tile([128, HPC, 512], f8, tag="wo")
                nc.sync.dma_start(t[:], wo_r[:, :, c * 512 : (c + 1) * 512])
                wo_sb.append(t)

            # ---- q/k/v projections: weight stationary, x moving ----
            # psum [128d, 4h*8b], accumulated over 32 e-tiles x (hi, lo)
            psq = ps_q.tile([128, HPC * B], fp32, tag="psq")
            psk = ps_k.tile([128, HPC * B], fp32, tag="psk")
            psv = ps_v.tile([128, HPC * B], fp32, tag="psv")
            for w_sb, ps in ((wq_sb, psq), (wk_sb, psk), (wv_sb, psv)):
                for ft in range(HPC):
                    dst = ps[:, ft * B : (ft + 1) * B]
                    for et in range(ET):
                        lhs = w_sb[:, et, ft * 128 : (ft + 1) * 128]
                        nc.tensor.matmul(
                            dst, lhs, x_sb[:, et, :B],
                            start=(et == 0), stop=False,
                        )
                        nc.tensor.matmul(
                            dst, lhs, x_sb[:, et, B:],
                            start=False, stop=(et == ET - 1),
                        )

            # ---- closed-form attention (scale QSC on q/k/v psums) ----
            km1 = small_pool.tile([128, HPC * B], fp32, tag="km1")
            nc.vector.tensor_scalar(
                km1[:], psk[:], 1.0 / QSC, -1.0,
                mybir.AluOpType.mult, mybir.AluOpType.add,
            )  # k - 1, exact scale
            vm1 = small_pool.tile([128, HPC * B], fp32, tag="vm1")
            nc.vector.tensor_scalar(
                vm1[:], psv[:], 1.0 / QSC, -1.0,
                mybir.AluOpType.mult, mybir.AluOpType.add,
            )  # v - 1
            qkm = small_pool.tile([128, HPC * B], fp32, tag="qkm")
            nc.vector.tensor_tensor(
                qkm[:], psq[:], km1[:], mybir.AluOpType.mult
            )  # q*(k-1), scale QSC
            # tt[1, 32] = sum_d q*(k-1) = s1 - s0 (scale QSC)
            ps_tt = ps_s.tile([1, HPC * B], fp32, tag="pstt")
            nc.tensor.matmul(ps_tt[:], ones_sb[:], qkm[:], start=True, stop=True)
            p_sb = small_pool.tile([1, HPC * B], fp32, tag="p")
            nc.scalar.activation(
                p_sb[:], ps_tt[:], mybir.ActivationFunctionType.Sigmoid,
                bias=bias_sb[:], scale=SCALE / QSC,
            )
            # broadcast p over partitions, folding the delta fp8 scale:
            # pb[128, 32] = p * DS
            ps_pb = ps_s.tile([128, HPC * B], fp32, tag="pspb")
            nc.tensor.matmul(ps_pb[:], c64_sb[:], p_sb[:], start=True, stop=True)
            dsc = small_pool.tile([128, HPC * B], fp32, tag="dsc")
            nc.vector.tensor_tensor(
                dsc[:], vm1[:], ps_pb[:], mybir.AluOpType.mult
            )  # delta * DS
            dhi = small_pool.tile([128, HPC * B], f8, tag="dhi")
            nc.vector.tensor_copy(dhi[:], dsc[:])
            dhf = small_pool.tile([128, HPC * B], fp32, tag="dhf")
            nc.vector.tensor_copy(dhf[:], dhi[:])
            dlo = small_pool.tile([128, HPC * B], f8, tag="dlo")
            nc.vector.tensor_tensor(
                dlo[:], dsc[:], dhf[:], mybir.AluOpType.subtract
            )

            # ---- out-proj: wo stationary, delta hi/lo moving; psum is
            # out^T [128e, 8b] per e-tile, scale OSC.  Rotating psum tiles
            # let chunk c+1 matmul while chunk c drains to SBUF ----
            out_sb = const_pool.tile([128, ET, B], fp32, tag="osb")
            for c in range(NWOC):
                pso = ps_o.tile([128, 4, B], fp32, tag="pso")
                for el in range(4):
                    dst = pso[:, el, :]
                    for ft in range(HPC):
                        lhs = wo_sb[c][:, ft, el * 128 : (el + 1) * 128]
                        nc.tensor.matmul(
                            dst, lhs, dhi[:, ft * B : (ft + 1) * B],
                            start=(ft == 0), stop=False,
                        )
                        nc.tensor.matmul(
                            dst, lhs, dlo[:, ft * B : (ft + 1) * B],
                            start=False, stop=(ft == HPC - 1),
                        )
                nc.vector.tensor_scalar_mul(
                    out_sb[:, c * 4 : (c + 1) * 4, :], pso[:], 1.0 / OSC
                )
            # two staggered writes on separate queues so neither SEQ/HWDGE
            # dispatch chain serializes behind the other's semaphore wait
            nc.sync.dma_start(out_r[:, : 4 * (NWOC - 1)], out_sb[:, : 4 * (NWOC - 1), :])
            nc.scalar.dma_start(
                out_r[:, 4 * (NWOC - 1) :], out_sb[:, 4 * (NWOC - 1) :, :]
            )

    nc.compile()
    return nc


def _get_program(mode=MODE):
    key = "nc_" + mode
    if key not in _CACHE:
        _CACHE[key] = _build_program()
    return _CACHE[key]


def _pack_pmajor(a, tiles):
    """[tiles*128, w] -> [128, tiles*w] partition-major contiguous."""
    w = a.shape[1]
    return np.ascontiguousarray(
        a.reshape(tiles, 128, w).transpose(1, 0, 2).reshape(128, tiles * w)
    )


def _shard_inputs(x, Wq, Wk, Wv, Wo, mode=MODE):
    import ml_dtypes

    f8 = ml_dtypes.float8_e4m3

    def q8(a):
        return np.clip(a, -240.0, 240.0).astype(f8)

    xt = x.reshape(B, E).T * XS  # [E, 8]
    xh = q8(xt)
    xl = q8(xt - xh.astype(np.float32))
    x2 = _pack_pmajor(np.concatenate([xh, xl], axis=1), ET)  # [128, 512]

    in_maps = []
    for c in range(N_CORES):
        rows = slice(c * F, (c + 1) * F)
        m = {
            "xt": x2,
            "wq_t": _pack_pmajor(q8(Wq[rows, :].T * WS), ET),
            "wk_t": _pack_pmajor(q8(Wk[rows, :].T * WS), ET),
            "wv_t": _pack_pmajor(q8(Wv[rows, :].T * WS), ET),
            "wo_t": _pack_pmajor(q8(Wo[:, rows].T * WS), HPC),
        }
        in_maps.append(m)
    return in_maps


def kernel(x, Wq, Wk, Wv, Wo, _trace=False, **_unused):
    from concourse.bass_utils import run_bass_kernel_spmd

    x = np.asarray(x, dtype=np.float32)
    Wq = np.asarray(Wq, dtype=np.float32)
    Wk = np.asarray(Wk, dtype=np.float32)
    Wv = np.asarray(Wv, dtype=np.float32)
    Wo = np.asarray(Wo, dtype=np.float32)

    nc = _get_program()
    in_maps = _shard_inputs(x, Wq, Wk, Wv, Wo)
    core_ids = list(range(N_CORES))

    def _run(trace):
        return run_bass_kernel_spmd(nc, in_maps, core_ids, trace=trace)

    res = None
    if _trace:
        try:
            res = _run(True)
        except Exception:
            # NTFF profiling hooks unavailable in this environment
            res = None
    if res is None:
        # transient device wedges (NRT_EXEC_UNIT_UNRECOVERABLE) heal after
        # a terminal-side reset; tear down the PJRT client and back off
        # before each retry
        import time as _time

        last = None
        for attempt in range(3):
            try:
                res = _run(False)
                break
            except Exception as e:
                last = e
                try:
                    import jax._src.xla_bridge as _xb

                    _xb._clear_backends()
                except Exception:
                    pass
                _time.sleep(15 * (attempt + 1))
        else:
            raise last
    _CACHE["last_results"] = res

    acc = np.zeros((B, E), np.float32)
    for r in res.results:
        # out_p[p, et, b] -> partial[b, et*128 + p]
        buf = np.asarray(r["out_p"], np.float32).reshape(128, ET, B)
        acc += buf.transpose(2, 1, 0).reshape(B, E)
    acc += Wo.sum(axis=1, dtype=np.float32)[None, :]
    return acc.reshape(B, 1, E)
